# revision 12
# baseline (speedup 1.0000x reference)
"""ConvPoolReadout GNN kernel for 8 TRN2 NeuronCores.

Strategy: graph-wise sharding (16 whole graphs per core, fully local).
The edge list is re-packed host-side (during sharding) into a dense
per-graph transposed adjacency At[s, d] (counts are exact small ints),
so all conv aggregation runs as fp32 matmuls on the TensorEngine.
Per-graph top-k is an exact 2-pass rank (pairwise count on DVE with an
index tiebreak key), pooled rows are emitted in score order via an fp16
one-hot permutation matmul, and readout = concat(mean, max) per graph.
"""
import os
import numpy as np

import concourse.bass as bass
import concourse.mybir as mybir
import concourse.tile as tile
from concourse import bacc
from concourse.bass_utils import run_bass_kernel_spmd
from concourse.masks import make_identity

B = 128          # graphs
NP = 512         # nodes per graph
F = 256          # feature dim (in == out)
DEG = 16
NCORES = 8
GPC = B // NCORES            # graphs per core = 16
NNC = GPC * NP               # nodes per core = 8192
K = 256                      # kept per graph
E = B * NP * DEG             # total edges
T = NP // 128                # node tiles per graph = 4
FC = F // 128                # feature chunks = 2

f32 = mybir.dt.float32
f16 = mybir.dt.float16
i16 = mybir.dt.int16


def _build(nc):
    featT = nc.dram_tensor("featT", [F, NNC], f32, kind="ExternalInput")
    At = nc.dram_tensor("At", [NNC, NP], f32, kind="ExternalInput")      # rows g*512+s, cols d
    sn_d = nc.dram_tensor("sn", [NNC], f32, kind="ExternalInput")
    dn_d = nc.dram_tensor("dn", [NNC], f32, kind="ExternalInput")
    W_d = nc.dram_tensor("W", [F, F], f32, kind="ExternalInput")
    b_d = nc.dram_tensor("b", [F], f32, kind="ExternalInput")
    pooled_d = nc.dram_tensor("pooled", [GPC * K, F], f32, kind="ExternalOutput")
    readout_d = nc.dram_tensor("readout", [GPC, 2 * F], f32, kind="ExternalOutput")

    with tile.TileContext(nc) as tc:
        with (
            tc.tile_pool(name="const", bufs=1) as cp,
            tc.tile_pool(name="sb", bufs=3) as sb,
            tc.tile_pool(name="sc", bufs=4) as scp,
            tc.tile_pool(name="jk", bufs=2) as jkp,
            tc.tile_pool(name="ps", bufs=2, space="PSUM") as ps,
            tc.tile_pool(name="ps1", bufs=1, space="PSUM") as ps1,
        ):
            # ---- persistent constants ----
            Wt = cp.tile([128, FC, F], f32, tag="W")           # W rows chunked
            nc.sync.dma_start(Wt[:], W_d.ap().rearrange("(c p) f -> p c f", p=128))
            b_rep = cp.tile([128, F], f32, tag="b")
            nc.sync.dma_start(
                b_rep[:], b_d.ap().rearrange("(o f) -> o f", o=1).to_broadcast([128, F])
            )
            iota_r16 = cp.tile([128, K], i16, tag="iotar_i")   # 0..255 on every partition
            nc.gpsimd.iota(iota_r16[:], pattern=[[1, K]], base=0, channel_multiplier=0)
            iota_r = cp.tile([128, K], f32, tag="iotar")
            nc.vector.tensor_copy(iota_r[:], iota_r16[:])
            idx_i16 = cp.tile([128, T], i16, tag="idx_i")      # node index t*128+p
            nc.gpsimd.iota(idx_i16[:], pattern=[[128, T]], base=0, channel_multiplier=1)
            idxf = cp.tile([128, T], f32, tag="idxf")
            nc.vector.tensor_copy(idxf[:], idx_i16[:])
            nc.vector.tensor_scalar_mul(idxf[:], idxf[:], 1.0 / 1024.0)  # i * 2^-10
            ident = cp.tile([128, 128], f32, tag="ident")
            make_identity(nc, ident[:])
            ones_col = cp.tile([128, 1], f32, tag="ones_col")
            nc.gpsimd.memset(ones_col[:], 1.0)
            ones_row = cp.tile([1, 128], f32, tag="ones_row")
            nc.gpsimd.memset(ones_row[:], 1.0)

            for g in range(GPC):
                gn = slice(g * NP, (g + 1) * NP)
                # ---- loads ----
                ft = sb.tile([128, FC, NP], f32, tag="ft")
                nc.sync.dma_start(
                    ft[:], featT.ap()[:, gn].rearrange("(c p) n -> p c n", p=128)
                )
                at = sb.tile([128, T, NP], f32, tag="at")
                nc.sync.dma_start(
                    at[:], At.ap()[gn, :].rearrange("(j p) d -> p j d", p=128)
                )
                snc = sb.tile([128, T], f32, tag="snc")
                nc.sync.dma_start(snc[:], sn_d.ap()[gn].rearrange("(t p) -> p t", p=128))
                dnc = sb.tile([128, T], f32, tag="dnc")
                nc.sync.dma_start(dnc[:], dn_d.ap()[gn].rearrange("(t p) -> p t", p=128))

                # ---- h = (X * sn) @ W  (scale before matmul, like the reference) ----
                snrep = sb.tile([128, NP], f32, tag="snrep")
                nc.sync.dma_start(
                    snrep[:],
                    sn_d.ap()[gn].rearrange("(o n) -> o n", o=1).to_broadcast([128, NP]),
                )
                ftn = sb.tile([128, FC, NP], f32, tag="ftn")
                for c in range(FC):
                    nc.vector.tensor_mul(ftn[:, c, :], ft[:, c, :], snrep[:])
                h = sb.tile([128, T, F], f32, tag="h")
                for t in range(T):
                    ph = ps1.tile([128, F], f32, tag="ph")
                    for c in range(FC):
                        nc.tensor.matmul(
                            ph[:],
                            ftn[:, c, t * 128:(t + 1) * 128],
                            Wt[:, c, :],
                            start=(c == 0),
                            stop=(c == FC - 1),
                        )
                    nc.scalar.activation(
                        h[:, t, :], ph[:], mybir.ActivationFunctionType.Copy
                    )

                # ---- conv1: out = relu(A@h * dn + b) ----
                out_sb = sb.tile([128, T, F], f32, tag="out")
                for t in range(T):
                    pagg = ps.tile([128, F], f32, tag="pagg")
                    for j in range(T):
                        nc.tensor.matmul(
                            pagg[:],
                            at[:, j, t * 128:(t + 1) * 128],
                            h[:, j, :],
                            start=(j == 0),
                            stop=(j == T - 1),
                        )
                    t1 = scp.tile([128, F], f32, tag="t1")
                    nc.vector.scalar_tensor_tensor(
                        t1[:], pagg[:], dnc[:, t:t + 1], b_rep[:],
                        op0=mybir.AluOpType.mult, op1=mybir.AluOpType.add,
                    )
                    nc.scalar.activation(
                        out_sb[:, t, :], t1[:], mybir.ActivationFunctionType.Relu
                    )

                # ---- conv2 on sf = out*sn; score = sum |out - (A@sf)*dn| ----
                sf = sb.tile([128, T, F], f32, tag="sf")
                for t in range(T):
                    nc.vector.tensor_scalar_mul(sf[:, t, :], out_sb[:, t, :], snc[:, t:t + 1])
                score_col = sb.tile([128, T], f32, tag="score_col")
                score_hi = sb.tile([128, T], f32, tag="score_hi")
                score_lo = sb.tile([128, T], f32, tag="score_lo")
                for t in range(T):
                    pagg2 = ps.tile([128, F], f32, tag="pagg2")
                    for j in range(T):
                        nc.tensor.matmul(
                            pagg2[:],
                            at[:, j, t * 128:(t + 1) * 128],
                            sf[:, j, :],
                            start=(j == 0),
                            stop=(j == T - 1),
                        )
                    dif = scp.tile([128, F], f32, tag="dif")
                    nc.vector.scalar_tensor_tensor(
                        dif[:], pagg2[:], dnc[:, t:t + 1], out_sb[:, t, :],
                        op0=mybir.AluOpType.mult, op1=mybir.AluOpType.subtract,
                    )
                    ab = scp.tile([128, F], f32, tag="ab")
                    nc.scalar.activation(ab[:], dif[:], mybir.ActivationFunctionType.Abs)
                    # near-exact sum: Dekker split |d| = hi + lo, hi on a 2^-10
                    # grid sums exactly in fp32; lo residuals are ~2^-11 scale
                    tmp = scp.tile([128, F], f32, tag="tmp")
                    nc.scalar.activation(
                        tmp[:], ab[:], mybir.ActivationFunctionType.Copy, bias=8192.0
                    )
                    hi = scp.tile([128, F], f32, tag="hi")
                    nc.scalar.activation(
                        hi[:], tmp[:], mybir.ActivationFunctionType.Copy, bias=-8192.0,
                        accum_out=score_hi[:, t:t + 1],
                    )
                    lo = scp.tile([128, F], f32, tag="lo")
                    nc.vector.tensor_sub(lo[:], ab[:], hi[:])
                    nc.vector.reduce_sum(
                        score_lo[:, t:t + 1], lo[:], axis=mybir.AxisListType.X
                    )

                nc.vector.tensor_add(score_col[:], score_hi[:], score_lo[:])

                # ---- rank pass 1: cnt_gt over scores ----
                psT = ps1.tile([1, NP], f32, tag="misc")
                for t in range(T):
                    nc.tensor.transpose(
                        psT[0:1, t * 128:(t + 1) * 128], score_col[:, t:t + 1], ident[:]
                    )
                sT = scp.tile([1, NP], f32, tag="sT")
                nc.vector.tensor_copy(sT[:], psT[:])
                srep_ps = ps1.tile([128, NP], f32, tag="pbc")
                nc.tensor.matmul(srep_ps[:], ones_row[:], sT[:], start=True, stop=True)
                srep = sb.tile([128, NP], f32, tag="srep")
                nc.scalar.activation(
                    srep[:], srep_ps[:], mybir.ActivationFunctionType.Copy
                )
                cgt = sb.tile([128, T], f32, tag="cgt")
                for t in range(T):
                    junk = jkp.tile([128, NP], f32, tag="junk")
                    nc.vector.tensor_scalar(
                        junk[:], srep[:], score_col[:, t:t + 1], 0.0,
                        op0=mybir.AluOpType.is_gt,
                        op1=mybir.AluOpType.add,
                        accum_out=cgt[:, t:t + 1],
                    )
                # ---- rank pass 2: stable rank via exact key = cnt_gt + i*2^-10 ----
                key_col = sb.tile([128, T], f32, tag="key_col")
                nc.vector.tensor_add(key_col[:], cgt[:], idxf[:])
                psT2 = ps1.tile([1, NP], f32, tag="misc")
                for t in range(T):
                    nc.tensor.transpose(
                        psT2[0:1, t * 128:(t + 1) * 128], key_col[:, t:t + 1], ident[:]
                    )
                kT = scp.tile([1, NP], f32, tag="kT")
                nc.vector.tensor_copy(kT[:], psT2[:])
                krep_ps = ps1.tile([128, NP], f32, tag="pbc")
                nc.tensor.matmul(krep_ps[:], ones_row[:], kT[:], start=True, stop=True)
                krep = sb.tile([128, NP], f32, tag="krep")
                nc.scalar.activation(
                    krep[:], krep_ps[:], mybir.ActivationFunctionType.Copy
                )
                rank = sb.tile([128, T], f32, tag="rank")
                for t in range(T):
                    junk2 = jkp.tile([128, NP], f32, tag="junk2")
                    nc.vector.tensor_scalar(
                        junk2[:], krep[:], key_col[:, t:t + 1], 0.0,
                        op0=mybir.AluOpType.is_lt,
                        op1=mybir.AluOpType.add,
                        accum_out=rank[:, t:t + 1],
                    )

                # ---- fp32 one-hot permutation: pooled[r] = out[i] where rank_i == r ----
                R = sb.tile([128, T, K], f16, tag="R")
                for t in range(T):
                    nc.vector.tensor_scalar(
                        R[:, t, :], iota_r[:], rank[:, t:t + 1], None,
                        op0=mybir.AluOpType.is_equal,
                    )
                oh = sb.tile([128, T, F], f16, tag="oh")
                for t in range(T):
                    nc.scalar.activation(
                        oh[:, t, :], out_sb[:, t, :], mybir.ActivationFunctionType.Copy
                    )
                pooled_sb = sb.tile([128, 2, F], f32, tag="pooled_sb")
                for q in range(2):
                    ppool = ps1.tile([128, F], f32, tag="ppool")
                    for t in range(T):
                        nc.tensor.matmul(
                            ppool[:],
                            R[:, t, q * 128:(q + 1) * 128],
                            oh[:, t, :],
                            start=(t == 0),
                            stop=(t == T - 1),
                        )
                    nc.scalar.activation(
                        pooled_sb[:, q, :], ppool[:], mybir.ActivationFunctionType.Copy
                    )
                nc.scalar.dma_start(
                    pooled_d.ap()[g * K:(g + 1) * K, :].rearrange("(q p) c -> p q c", p=128),
                    pooled_sb[:],
                )

                # ---- readout mean = column sums of pooled / K ----
                pmean = ps1.tile([1, F], f32, tag="misc")
                for q in range(2):
                    nc.tensor.matmul(
                        pmean[:],
                        ones_col[:],
                        pooled_sb[:, q, :],
                        start=(q == 0),
                        stop=(q == 1),
                    )
                mrow = sb.tile([1, F], f32, tag="mrow")
                nc.scalar.activation(
                    mrow[:], pmean[:], mybir.ActivationFunctionType.Copy, scale=1.0 / K
                )
                nc.gpsimd.dma_start(readout_d.ap()[g, 0:F].rearrange("(o f) -> o f", o=1), mrow[:])

                # ---- readout max: PE-transpose pooled then reduce over rows ----
                for cb in range(2):
                    mxc = sb.tile([128, 2], f32, tag="mxc")
                    for q in range(2):
                        ptr = ps1.tile([128, 128], f32, tag="misc")
                        nc.tensor.transpose(
                            ptr[:], pooled_sb[:, q, cb * 128:(cb + 1) * 128], ident[:]
                        )
                        nc.vector.reduce_max(
                            mxc[:, q:q + 1], ptr[:], axis=mybir.AxisListType.X
                        )
                    mx = sb.tile([128, 1], f32, tag="mx")
                    nc.vector.tensor_max(mx[:], mxc[:, 0:1], mxc[:, 1:2])
                    nc.gpsimd.dma_start(
                        readout_d.ap()[g, F + cb * 128:F + (cb + 1) * 128]
                        .rearrange("(p o) -> p o", o=1),
                        mx[:],
                    )
    return nc


def _jax_invsqrt(deg):
    """Bit-match the reference's jnp.power(clip(deg,1), -0.5) on CPU."""
    try:
        import jax
        cpu = jax.local_devices(backend="cpu")[0]
        with jax.default_device(cpu):
            import jax.numpy as jnp
            return np.asarray(
                jnp.power(jnp.clip(jax.device_put(deg, cpu), 1.0), -0.5), np.float32
            )
    except Exception:
        return (np.clip(deg, 1.0, None) ** -0.5).astype(np.float32)


_NC_CACHE = {}


def _get_nc():
    if "nc" not in _NC_CACHE:
        nc = bacc.Bacc("TRN2", target_bir_lowering=False, debug=False, num_devices=NCORES)
        _build(nc)
        nc.compile()
        _NC_CACHE["nc"] = nc
    return _NC_CACHE["nc"]


def kernel(feat, e_feat, W, b, src, dst):
    feat = np.asarray(feat, np.float32)
    e_feat = np.asarray(e_feat, np.float32)
    W = np.asarray(W, np.float32)
    b = np.asarray(b, np.float32)
    src = np.asarray(src)
    dst = np.asarray(dst)
    N = B * NP

    out_deg = np.bincount(src, minlength=N).astype(np.float32)
    in_deg = np.bincount(dst, minlength=N).astype(np.float32)
    sn = _jax_invsqrt(out_deg)
    dn = _jax_invsqrt(in_deg)

    # dense transposed adjacency per graph: At[g*512+s, d] = sum of e_feat over edges (s->d)
    g = src // NP
    lin = (g.astype(np.int64) * NP + (src % NP)) * NP + (dst % NP)
    At_full = np.bincount(lin, weights=e_feat.astype(np.float64), minlength=B * NP * NP)
    At_full = At_full.reshape(B * NP, NP).astype(np.float32)

    featT = np.ascontiguousarray(feat.T)

    nc = _get_nc()
    in_maps = []
    for c in range(NCORES):
        nsl = slice(c * NNC, (c + 1) * NNC)
        in_maps.append({
            "featT": np.ascontiguousarray(featT[:, nsl]),
            "At": np.ascontiguousarray(At_full[nsl]),
            "sn": np.ascontiguousarray(sn[nsl]),
            "dn": np.ascontiguousarray(dn[nsl]),
            "W": W,
            "b": b,
        })
    trace = bool(int(os.environ.get("KERNEL_TRACE", "0")))
    if trace:
        try:
            import antenv.axon_hooks  # noqa: F401
        except ImportError:
            import sys
            import types
            from trn_agent_boot.trn_boot import _ntff_profile_via_ctypes
            _h = _ntff_profile_via_ctypes("/opt/axon/libaxon_pjrt.so")
            mod = types.ModuleType("antenv.axon_hooks")
            mod.get_axon_ntff_profile_hook = lambda: _h
            mod.set_axon_ntff_profile_hook = lambda hook: None
            sys.modules["antenv.axon_hooks"] = mod
    res = run_bass_kernel_spmd(
        nc, in_maps, core_ids=list(range(NCORES)), trace=trace,
        tmpdir=os.environ.get("KERNEL_TRACE_DIR") or None,
    )
    if trace and res.exec_time_ns is not None:
        print(f"HW exec time: {res.exec_time_ns} ns")
    outs = res.results
    pooled = np.concatenate([outs[c]["pooled"] for c in range(NCORES)], axis=0)
    readout = np.concatenate([outs[c]["readout"] for c in range(NCORES)], axis=0)
    return pooled, readout


# revision 13
# speedup vs baseline: 1.3209x; 1.3209x over previous
"""ConvPoolReadout GNN kernel for 8 TRN2 NeuronCores.

Strategy: graph-wise sharding (16 whole graphs per core, fully local).
The edge list is re-packed host-side (during sharding) into a dense
per-graph transposed adjacency At[s, d] (counts are exact small ints),
so all conv aggregation runs as fp32 matmuls on the TensorEngine.
Per-graph top-k is an exact 2-pass rank (pairwise count on DVE with an
index tiebreak key), pooled rows are emitted in score order via an fp16
one-hot permutation matmul, and readout = concat(mean, max) per graph.
"""
import os
import numpy as np

import concourse.bass as bass
import concourse.mybir as mybir
import concourse.tile as tile
from concourse import bacc
from concourse.bass_utils import run_bass_kernel_spmd
from concourse.masks import make_identity

B = 128          # graphs
NP = 512         # nodes per graph
F = 256          # feature dim (in == out)
DEG = 16
NCORES = 8
GPC = B // NCORES            # graphs per core = 16
NNC = GPC * NP               # nodes per core = 8192
K = 256                      # kept per graph
E = B * NP * DEG             # total edges
T = NP // 128                # node tiles per graph = 4
FC = F // 128                # feature chunks = 2

f32 = mybir.dt.float32
f16 = mybir.dt.float16
i16 = mybir.dt.int16


def _build(nc):
    featT = nc.dram_tensor("featT", [F, NNC], f32, kind="ExternalInput")
    At = nc.dram_tensor("At", [NNC, NP], f32, kind="ExternalInput")      # rows g*512+s, cols d
    sn_d = nc.dram_tensor("sn", [NNC], f32, kind="ExternalInput")
    dn_d = nc.dram_tensor("dn", [NNC], f32, kind="ExternalInput")
    W_d = nc.dram_tensor("W", [F, F], f32, kind="ExternalInput")
    b_d = nc.dram_tensor("b", [F], f32, kind="ExternalInput")
    pooled_d = nc.dram_tensor("pooled", [GPC * K, F], f32, kind="ExternalOutput")
    readout_d = nc.dram_tensor("readout", [GPC, 2 * F], f32, kind="ExternalOutput")
    srk_d = nc.dram_tensor("srk", [GPC, 2, NP], f32, kind="Internal")

    with tile.TileContext(nc) as tc:
        with (
            tc.tile_pool(name="const", bufs=1) as cp,
            tc.tile_pool(name="sb", bufs=3) as sb,
            tc.tile_pool(name="sc", bufs=4) as scp,
            tc.tile_pool(name="jk", bufs=2) as jkp,
            tc.tile_pool(name="ps", bufs=2, space="PSUM") as ps,
            tc.tile_pool(name="ps1", bufs=1, space="PSUM") as ps1,
        ):
            # ---- persistent constants ----
            Wt = cp.tile([128, FC, F], f32, tag="W")           # W rows chunked
            nc.sync.dma_start(Wt[:], W_d.ap().rearrange("(c p) f -> p c f", p=128))
            b_rep = cp.tile([128, F], f32, tag="b")
            nc.sync.dma_start(
                b_rep[:], b_d.ap().rearrange("(o f) -> o f", o=1).to_broadcast([128, F])
            )
            iota_r16 = cp.tile([128, K], i16, tag="iotar_i")   # 0..255 on every partition
            nc.gpsimd.iota(iota_r16[:], pattern=[[1, K]], base=0, channel_multiplier=0)
            iota_r = cp.tile([128, K], f32, tag="iotar")
            nc.vector.tensor_copy(iota_r[:], iota_r16[:])
            idx_i16 = cp.tile([128, T], i16, tag="idx_i")      # node index t*128+p
            nc.gpsimd.iota(idx_i16[:], pattern=[[128, T]], base=0, channel_multiplier=1)
            idxf = cp.tile([128, T], f32, tag="idxf")
            nc.vector.tensor_copy(idxf[:], idx_i16[:])
            nc.vector.tensor_scalar_mul(idxf[:], idxf[:], 1.0 / 1024.0)  # i * 2^-10
            ident = cp.tile([128, 128], f32, tag="ident")
            make_identity(nc, ident[:])
            ones_col = cp.tile([128, 1], f32, tag="ones_col")
            nc.gpsimd.memset(ones_col[:], 1.0)
            ones_row = cp.tile([1, 128], f32, tag="ones_row")
            nc.gpsimd.memset(ones_row[:], 1.0)

            for g in range(GPC):
                gn = slice(g * NP, (g + 1) * NP)
                # ---- loads ----
                ft = sb.tile([128, FC, NP], f32, tag="ft")
                nc.sync.dma_start(
                    ft[:], featT.ap()[:, gn].rearrange("(c p) n -> p c n", p=128)
                )
                at = sb.tile([128, T, NP], f32, tag="at")
                nc.sync.dma_start(
                    at[:], At.ap()[gn, :].rearrange("(j p) d -> p j d", p=128)
                )
                snc = sb.tile([128, T], f32, tag="snc")
                nc.sync.dma_start(snc[:], sn_d.ap()[gn].rearrange("(t p) -> p t", p=128))
                dnc = sb.tile([128, T], f32, tag="dnc")
                nc.sync.dma_start(dnc[:], dn_d.ap()[gn].rearrange("(t p) -> p t", p=128))

                # ---- h = (X * sn) @ W  (scale before matmul, like the reference) ----
                snrep = sb.tile([128, NP], f32, tag="snrep")
                nc.sync.dma_start(
                    snrep[:],
                    sn_d.ap()[gn].rearrange("(o n) -> o n", o=1).to_broadcast([128, NP]),
                )
                ftn = sb.tile([128, FC, NP], f32, tag="ftn")
                for c in range(FC):
                    nc.vector.tensor_mul(ftn[:, c, :], ft[:, c, :], snrep[:])
                h = sb.tile([128, T, F], f32, tag="h")
                for t in range(T):
                    ph = ps.tile([128, F], f32, tag="ph")
                    for c in range(FC):
                        nc.tensor.matmul(
                            ph[:],
                            ftn[:, c, t * 128:(t + 1) * 128],
                            Wt[:, c, :],
                            start=(c == 0),
                            stop=(c == FC - 1),
                        )
                    nc.scalar.activation(
                        h[:, t, :], ph[:], mybir.ActivationFunctionType.Copy
                    )

                # ---- conv1: out = relu(A@h * dn + b) ----
                out_sb = sb.tile([128, T, F], f32, tag="out")
                for t in range(T):
                    pagg = ps.tile([128, F], f32, tag="pagg")
                    for j in range(T):
                        nc.tensor.matmul(
                            pagg[:],
                            at[:, j, t * 128:(t + 1) * 128],
                            h[:, j, :],
                            start=(j == 0),
                            stop=(j == T - 1),
                        )
                    t1 = scp.tile([128, F], f32, tag="t1")
                    nc.vector.scalar_tensor_tensor(
                        t1[:], pagg[:], dnc[:, t:t + 1], b_rep[:],
                        op0=mybir.AluOpType.mult, op1=mybir.AluOpType.add,
                    )
                    nc.scalar.activation(
                        out_sb[:, t, :], t1[:], mybir.ActivationFunctionType.Relu
                    )

                # ---- conv2 on sf = out*sn; score = sum |out - (A@sf)*dn| ----
                sf = sb.tile([128, T, F], f32, tag="sf")
                for t in range(T):
                    nc.vector.tensor_scalar_mul(sf[:, t, :], out_sb[:, t, :], snc[:, t:t + 1])
                score_col = sb.tile([128, T], f32, tag="score_col")
                score_hi = sb.tile([128, T], f32, tag="score_hi")
                score_lo = sb.tile([128, T], f32, tag="score_lo")
                for t in range(T):
                    pagg2 = ps.tile([128, F], f32, tag="pagg2")
                    for j in range(T):
                        nc.tensor.matmul(
                            pagg2[:],
                            at[:, j, t * 128:(t + 1) * 128],
                            sf[:, j, :],
                            start=(j == 0),
                            stop=(j == T - 1),
                        )
                    dif = scp.tile([128, F], f32, tag="dif")
                    nc.vector.scalar_tensor_tensor(
                        dif[:], pagg2[:], dnc[:, t:t + 1], out_sb[:, t, :],
                        op0=mybir.AluOpType.mult, op1=mybir.AluOpType.subtract,
                    )
                    ab = scp.tile([128, F], f32, tag="ab")
                    nc.scalar.activation(ab[:], dif[:], mybir.ActivationFunctionType.Abs)
                    # near-exact sum: Dekker split |d| = hi + lo, hi on a 2^-10
                    # grid sums exactly in fp32; lo residuals are ~2^-11 scale
                    tmp = scp.tile([128, F], f32, tag="tmp")
                    nc.scalar.activation(
                        tmp[:], ab[:], mybir.ActivationFunctionType.Copy, bias=8192.0
                    )
                    hi = scp.tile([128, F], f32, tag="hi")
                    nc.scalar.activation(
                        hi[:], tmp[:], mybir.ActivationFunctionType.Copy, bias=-8192.0,
                        accum_out=score_hi[:, t:t + 1],
                    )
                    lo = scp.tile([128, F], f32, tag="lo")
                    nc.vector.tensor_sub(lo[:], ab[:], hi[:])
                    nc.vector.reduce_sum(
                        score_lo[:, t:t + 1], lo[:], axis=mybir.AxisListType.X
                    )

                nc.vector.tensor_add(score_col[:], score_hi[:], score_lo[:])

                # ---- rank pass 1: cnt_gt over scores ----
                nc.gpsimd.dma_start(
                    srk_d.ap()[g, 0].rearrange("(t p) -> p t", p=128), score_col[:]
                )
                srep = sb.tile([128, NP], f32, tag="srep")
                nc.gpsimd.dma_start(
                    srep[:],
                    srk_d.ap()[g, 0].rearrange("(o n) -> o n", o=1).to_broadcast([128, NP]),
                )
                cgt = sb.tile([128, T], f32, tag="cgt")
                for t in range(T):
                    junk = jkp.tile([128, NP], f32, tag="junk")
                    nc.vector.tensor_scalar(
                        junk[:], srep[:], score_col[:, t:t + 1], 0.0,
                        op0=mybir.AluOpType.is_gt,
                        op1=mybir.AluOpType.add,
                        accum_out=cgt[:, t:t + 1],
                    )
                # ---- rank pass 2: stable rank via exact key = cnt_gt + i*2^-10 ----
                key_col = sb.tile([128, T], f32, tag="key_col")
                nc.vector.tensor_add(key_col[:], cgt[:], idxf[:])
                krep = sb.tile([128, NP], f32, tag="krep")
                nc.gpsimd.dma_start(
                    srk_d.ap()[g, 1].rearrange("(t p) -> p t", p=128), key_col[:]
                )
                nc.gpsimd.dma_start(
                    krep[:],
                    srk_d.ap()[g, 1].rearrange("(o n) -> o n", o=1).to_broadcast([128, NP]),
                )
                rank = sb.tile([128, T], f32, tag="rank")
                for t in range(T):
                    junk2 = jkp.tile([128, NP], f32, tag="junk2")
                    nc.vector.tensor_scalar(
                        junk2[:], krep[:], key_col[:, t:t + 1], 0.0,
                        op0=mybir.AluOpType.is_lt,
                        op1=mybir.AluOpType.add,
                        accum_out=rank[:, t:t + 1],
                    )

                # ---- fp32 one-hot permutation: pooled[r] = out[i] where rank_i == r ----
                R = sb.tile([128, T, K], f16, tag="R")
                for t in range(T):
                    nc.vector.tensor_scalar(
                        R[:, t, :], iota_r[:], rank[:, t:t + 1], None,
                        op0=mybir.AluOpType.is_equal,
                    )
                oh = sb.tile([128, T, F], f16, tag="oh")
                for t in range(T):
                    nc.scalar.activation(
                        oh[:, t, :], out_sb[:, t, :], mybir.ActivationFunctionType.Copy
                    )
                pooled_sb = sb.tile([128, 2, F], f32, tag="pooled_sb")
                for q in range(2):
                    ppool = ps1.tile([128, F], f32, tag="ppool")
                    for t in range(T):
                        nc.tensor.matmul(
                            ppool[:],
                            R[:, t, q * 128:(q + 1) * 128],
                            oh[:, t, :],
                            start=(t == 0),
                            stop=(t == T - 1),
                        )
                    nc.scalar.activation(
                        pooled_sb[:, q, :], ppool[:], mybir.ActivationFunctionType.Copy
                    )
                nc.scalar.dma_start(
                    pooled_d.ap()[g * K:(g + 1) * K, :].rearrange("(q p) c -> p q c", p=128),
                    pooled_sb[:],
                )

                # ---- readout mean = column sums of pooled / K ----
                pmean = ps1.tile([1, F], f32, tag="misc")
                for q in range(2):
                    nc.tensor.matmul(
                        pmean[:],
                        ones_col[:],
                        pooled_sb[:, q, :],
                        start=(q == 0),
                        stop=(q == 1),
                    )
                mrow = sb.tile([1, F], f32, tag="mrow")
                nc.scalar.activation(
                    mrow[:], pmean[:], mybir.ActivationFunctionType.Copy, scale=1.0 / K
                )
                nc.gpsimd.dma_start(readout_d.ap()[g, 0:F].rearrange("(o f) -> o f", o=1), mrow[:])

                # ---- readout max: PE-transpose pooled then reduce over rows ----
                for cb in range(2):
                    mxc = sb.tile([128, 2], f32, tag="mxc")
                    for q in range(2):
                        ptr = ps1.tile([128, 128], f32, tag="misc")
                        nc.tensor.transpose(
                            ptr[:], pooled_sb[:, q, cb * 128:(cb + 1) * 128], ident[:]
                        )
                        nc.vector.reduce_max(
                            mxc[:, q:q + 1], ptr[:], axis=mybir.AxisListType.X
                        )
                    mx = sb.tile([128, 1], f32, tag="mx")
                    nc.vector.tensor_max(mx[:], mxc[:, 0:1], mxc[:, 1:2])
                    nc.gpsimd.dma_start(
                        readout_d.ap()[g, F + cb * 128:F + (cb + 1) * 128]
                        .rearrange("(p o) -> p o", o=1),
                        mx[:],
                    )
    return nc


def _jax_invsqrt(deg):
    """Bit-match the reference's jnp.power(clip(deg,1), -0.5) on CPU."""
    try:
        import jax
        cpu = jax.local_devices(backend="cpu")[0]
        with jax.default_device(cpu):
            import jax.numpy as jnp
            return np.asarray(
                jnp.power(jnp.clip(jax.device_put(deg, cpu), 1.0), -0.5), np.float32
            )
    except Exception:
        return (np.clip(deg, 1.0, None) ** -0.5).astype(np.float32)


_NC_CACHE = {}


def _get_nc():
    if "nc" not in _NC_CACHE:
        nc = bacc.Bacc("TRN2", target_bir_lowering=False, debug=False, num_devices=NCORES)
        _build(nc)
        nc.compile()
        _NC_CACHE["nc"] = nc
    return _NC_CACHE["nc"]


def kernel(feat, e_feat, W, b, src, dst):
    feat = np.asarray(feat, np.float32)
    e_feat = np.asarray(e_feat, np.float32)
    W = np.asarray(W, np.float32)
    b = np.asarray(b, np.float32)
    src = np.asarray(src)
    dst = np.asarray(dst)
    N = B * NP

    out_deg = np.bincount(src, minlength=N).astype(np.float32)
    in_deg = np.bincount(dst, minlength=N).astype(np.float32)
    sn = _jax_invsqrt(out_deg)
    dn = _jax_invsqrt(in_deg)

    # dense transposed adjacency per graph: At[g*512+s, d] = sum of e_feat over edges (s->d)
    g = src // NP
    lin = (g.astype(np.int64) * NP + (src % NP)) * NP + (dst % NP)
    At_full = np.bincount(lin, weights=e_feat.astype(np.float64), minlength=B * NP * NP)
    At_full = At_full.reshape(B * NP, NP).astype(np.float32)

    featT = np.ascontiguousarray(feat.T)

    nc = _get_nc()
    in_maps = []
    for c in range(NCORES):
        nsl = slice(c * NNC, (c + 1) * NNC)
        in_maps.append({
            "featT": np.ascontiguousarray(featT[:, nsl]),
            "At": np.ascontiguousarray(At_full[nsl]),
            "sn": np.ascontiguousarray(sn[nsl]),
            "dn": np.ascontiguousarray(dn[nsl]),
            "W": W,
            "b": b,
        })
    trace = bool(int(os.environ.get("KERNEL_TRACE", "0")))
    if trace:
        try:
            import antenv.axon_hooks  # noqa: F401
        except ImportError:
            import sys
            import types
            from trn_agent_boot.trn_boot import _ntff_profile_via_ctypes
            _h = _ntff_profile_via_ctypes("/opt/axon/libaxon_pjrt.so")
            mod = types.ModuleType("antenv.axon_hooks")
            mod.get_axon_ntff_profile_hook = lambda: _h
            mod.set_axon_ntff_profile_hook = lambda hook: None
            sys.modules["antenv.axon_hooks"] = mod
    res = run_bass_kernel_spmd(
        nc, in_maps, core_ids=list(range(NCORES)), trace=trace,
        tmpdir=os.environ.get("KERNEL_TRACE_DIR") or None,
    )
    if trace and res.exec_time_ns is not None:
        print(f"HW exec time: {res.exec_time_ns} ns")
    outs = res.results
    pooled = np.concatenate([outs[c]["pooled"] for c in range(NCORES)], axis=0)
    readout = np.concatenate([outs[c]["readout"] for c in range(NCORES)], axis=0)
    return pooled, readout


# revision 14
# speedup vs baseline: 1.8240x; 1.3809x over previous
"""ConvPoolReadout GNN kernel for 8 TRN2 NeuronCores.

Strategy: graph-wise sharding (16 whole graphs per core, fully local).
The edge list is re-packed host-side (during sharding) into a dense
per-graph transposed adjacency At[s, d] (counts are exact small ints),
so all conv aggregation runs as fp32 matmuls on the TensorEngine.
Per-graph top-k is an exact 2-pass rank (pairwise count on DVE with an
index tiebreak key), pooled rows are emitted in score order via an fp16
one-hot permutation matmul, and readout = concat(mean, max) per graph.
"""
import os
import numpy as np

import concourse.bass as bass
import concourse.mybir as mybir
import concourse.tile as tile
from concourse import bacc
from concourse.bass_utils import run_bass_kernel_spmd
from concourse.masks import make_identity

B = 128          # graphs
NP = 512         # nodes per graph
F = 256          # feature dim (in == out)
DEG = 16
NCORES = 8
GPC = B // NCORES            # graphs per core = 16
NNC = GPC * NP               # nodes per core = 8192
K = 256                      # kept per graph
E = B * NP * DEG             # total edges
T = NP // 128                # node tiles per graph = 4
FC = F // 128                # feature chunks = 2

f32 = mybir.dt.float32
f16 = mybir.dt.float16
i16 = mybir.dt.int16


def _build(nc):
    featT = nc.dram_tensor("featT", [F, NNC], f32, kind="ExternalInput")
    At = nc.dram_tensor("At", [NNC, NP], f32, kind="ExternalInput")      # rows g*512+s, cols d
    sn_d = nc.dram_tensor("sn", [NNC], f32, kind="ExternalInput")
    dn_d = nc.dram_tensor("dn", [NNC], f32, kind="ExternalInput")
    W_d = nc.dram_tensor("W", [F, F], f32, kind="ExternalInput")
    b_d = nc.dram_tensor("b", [F], f32, kind="ExternalInput")
    pooled_d = nc.dram_tensor("pooled", [GPC * K, F], f32, kind="ExternalOutput")
    readout_d = nc.dram_tensor("readout", [GPC, 2 * F], f32, kind="ExternalOutput")
    srk_d = nc.dram_tensor("srk", [GPC, 2, NP], f32, kind="Internal")

    with tile.TileContext(nc) as tc:
        with (
            tc.tile_pool(name="const", bufs=1) as cp,
            tc.tile_pool(name="sb", bufs=3) as sb,
            tc.tile_pool(name="sc", bufs=4) as scp,
            tc.tile_pool(name="jk", bufs=4) as jkp,
            tc.tile_pool(name="ps", bufs=2, space="PSUM") as ps,
            tc.tile_pool(name="ps1", bufs=1, space="PSUM") as ps1,
        ):
            # ---- persistent constants ----
            Wt = cp.tile([128, FC, F], f32, tag="W")           # W rows chunked
            nc.sync.dma_start(Wt[:], W_d.ap().rearrange("(c p) f -> p c f", p=128))
            b_rep = cp.tile([128, F], f32, tag="b")
            nc.sync.dma_start(
                b_rep[:], b_d.ap().rearrange("(o f) -> o f", o=1).to_broadcast([128, F])
            )
            iota_r16 = cp.tile([128, K], i16, tag="iotar_i")   # 0..255 on every partition
            nc.gpsimd.iota(iota_r16[:], pattern=[[1, K]], base=0, channel_multiplier=0)
            iota_r = cp.tile([128, K], f32, tag="iotar")
            nc.vector.tensor_copy(iota_r[:], iota_r16[:])
            idx_i16 = cp.tile([128, T], i16, tag="idx_i")      # node index t*128+p
            nc.gpsimd.iota(idx_i16[:], pattern=[[128, T]], base=0, channel_multiplier=1)
            idxf = cp.tile([128, T], f32, tag="idxf")
            nc.vector.tensor_copy(idxf[:], idx_i16[:])
            nc.vector.tensor_scalar_mul(idxf[:], idxf[:], 1.0 / 1024.0)  # i * 2^-10
            ident = cp.tile([128, 128], f32, tag="ident")
            make_identity(nc, ident[:])
            ones_col = cp.tile([128, 1], f32, tag="ones_col")
            nc.gpsimd.memset(ones_col[:], 1.0)
            ones_row = cp.tile([1, 128], f32, tag="ones_row")
            nc.gpsimd.memset(ones_row[:], 1.0)

            for g in range(GPC):
                gn = slice(g * NP, (g + 1) * NP)
                # ---- loads ----
                ft = sb.tile([128, FC, NP], f32, tag="ft")
                nc.sync.dma_start(
                    ft[:], featT.ap()[:, gn].rearrange("(c p) n -> p c n", p=128)
                )
                at = sb.tile([128, T, NP], f32, tag="at")
                nc.sync.dma_start(
                    at[:], At.ap()[gn, :].rearrange("(j p) d -> p j d", p=128)
                )
                snc = sb.tile([128, T], f32, tag="snc")
                nc.sync.dma_start(snc[:], sn_d.ap()[gn].rearrange("(t p) -> p t", p=128))
                dnc = sb.tile([128, T], f32, tag="dnc")
                nc.sync.dma_start(dnc[:], dn_d.ap()[gn].rearrange("(t p) -> p t", p=128))

                # ---- h = (X * sn) @ W  (scale before matmul, like the reference) ----
                snrep = sb.tile([128, NP], f32, tag="snrep")
                nc.sync.dma_start(
                    snrep[:],
                    sn_d.ap()[gn].rearrange("(o n) -> o n", o=1).to_broadcast([128, NP]),
                )
                ftn = sb.tile([128, FC, NP], f32, tag="ftn")
                for c in range(FC):
                    nc.vector.tensor_mul(ftn[:, c, :], ft[:, c, :], snrep[:])
                h = sb.tile([128, T, F], f32, tag="h")
                for t in range(T):
                    ph = ps.tile([128, F], f32, tag="ph")
                    for c in range(FC):
                        nc.tensor.matmul(
                            ph[:],
                            ftn[:, c, t * 128:(t + 1) * 128],
                            Wt[:, c, :],
                            start=(c == 0),
                            stop=(c == FC - 1),
                        )
                    nc.scalar.activation(
                        h[:, t, :], ph[:], mybir.ActivationFunctionType.Copy
                    )

                # ---- conv1: out = relu(A@h * dn + b) ----
                out_sb = sb.tile([128, T, F], f32, tag="out")
                for t in range(T):
                    pagg = ps.tile([128, F], f32, tag="pagg")
                    for j in range(T):
                        nc.tensor.matmul(
                            pagg[:],
                            at[:, j, t * 128:(t + 1) * 128],
                            h[:, j, :],
                            start=(j == 0),
                            stop=(j == T - 1),
                        )
                    t1 = scp.tile([128, F], f32, tag="t1")
                    nc.vector.scalar_tensor_tensor(
                        t1[:], pagg[:], dnc[:, t:t + 1], b_rep[:],
                        op0=mybir.AluOpType.mult, op1=mybir.AluOpType.add,
                    )
                    nc.scalar.activation(
                        out_sb[:, t, :], t1[:], mybir.ActivationFunctionType.Relu
                    )

                # ---- conv2 on sf = out*sn; score = sum |out - (A@sf)*dn| ----
                sf = sb.tile([128, T, F], f32, tag="sf")
                for t in range(T):
                    nc.vector.tensor_scalar_mul(sf[:, t, :], out_sb[:, t, :], snc[:, t:t + 1])
                score_col = sb.tile([128, T], f32, tag="score_col")
                score_hi = sb.tile([128, T], f32, tag="score_hi")
                score_lo = sb.tile([128, T], f32, tag="score_lo")
                for t in range(T):
                    pagg2 = ps.tile([128, F], f32, tag="pagg2")
                    for j in range(T):
                        nc.tensor.matmul(
                            pagg2[:],
                            at[:, j, t * 128:(t + 1) * 128],
                            sf[:, j, :],
                            start=(j == 0),
                            stop=(j == T - 1),
                        )
                    dif = scp.tile([128, F], f32, tag="dif")
                    nc.vector.scalar_tensor_tensor(
                        dif[:], pagg2[:], dnc[:, t:t + 1], out_sb[:, t, :],
                        op0=mybir.AluOpType.mult, op1=mybir.AluOpType.subtract,
                    )
                    ab = scp.tile([128, F], f32, tag="ab")
                    nc.scalar.activation(ab[:], dif[:], mybir.ActivationFunctionType.Abs)
                    # near-exact sum: Dekker split |d| = hi + lo, hi on a 2^-10
                    # grid sums exactly in fp32; lo residuals are ~2^-11 scale
                    tmp = scp.tile([128, F], f32, tag="tmp")
                    nc.scalar.activation(
                        tmp[:], ab[:], mybir.ActivationFunctionType.Copy, bias=8192.0
                    )
                    hi = scp.tile([128, F], f32, tag="hi")
                    nc.scalar.activation(
                        hi[:], tmp[:], mybir.ActivationFunctionType.Copy, bias=-8192.0,
                        accum_out=score_hi[:, t:t + 1],
                    )
                    lo = scp.tile([128, F], f32, tag="lo")
                    nc.vector.tensor_sub(lo[:], ab[:], hi[:])
                    nc.vector.reduce_sum(
                        score_lo[:, t:t + 1], lo[:], axis=mybir.AxisListType.X
                    )

                nc.vector.tensor_add(score_col[:], score_hi[:], score_lo[:])

                # ---- rank pass 1: cnt_gt over scores ----
                nc.gpsimd.dma_start(
                    srk_d.ap()[g, 0].rearrange("(t p) -> p t", p=128), score_col[:]
                )
                srep = sb.tile([128, NP], f32, tag="srep")
                nc.gpsimd.dma_start(
                    srep[:],
                    srk_d.ap()[g, 0].rearrange("(o n) -> o n", o=1).to_broadcast([128, NP]),
                )
                cgt = sb.tile([128, T], f32, tag="cgt")
                for t in range(T):
                    junk = jkp.tile([128, NP], f32, tag="junk")
                    nc.vector.tensor_scalar(
                        junk[:], srep[:], score_col[:, t:t + 1], 0.0,
                        op0=mybir.AluOpType.is_gt,
                        op1=mybir.AluOpType.add,
                        accum_out=cgt[:, t:t + 1],
                    )
                # ---- rank pass 2: stable rank via exact key = cnt_gt + i*2^-10 ----
                key_col = sb.tile([128, T], f32, tag="key_col")
                nc.vector.tensor_add(key_col[:], cgt[:], idxf[:])
                krep = sb.tile([128, NP], f32, tag="krep")
                nc.gpsimd.dma_start(
                    srk_d.ap()[g, 1].rearrange("(t p) -> p t", p=128), key_col[:]
                )
                nc.gpsimd.dma_start(
                    krep[:],
                    srk_d.ap()[g, 1].rearrange("(o n) -> o n", o=1).to_broadcast([128, NP]),
                )
                rank = sb.tile([128, T], f32, tag="rank")
                for t in range(T):
                    junk2 = jkp.tile([128, NP], f32, tag="junk2")
                    nc.vector.tensor_scalar(
                        junk2[:], krep[:], key_col[:, t:t + 1], 0.0,
                        op0=mybir.AluOpType.is_lt,
                        op1=mybir.AluOpType.add,
                        accum_out=rank[:, t:t + 1],
                    )

                # ---- fp32 one-hot permutation: pooled[r] = out[i] where rank_i == r ----
                R = sb.tile([128, T, K], f16, tag="R")
                for t in range(T):
                    nc.vector.tensor_scalar(
                        R[:, t, :], iota_r[:], rank[:, t:t + 1], None,
                        op0=mybir.AluOpType.is_equal,
                    )
                oh = sb.tile([128, T, F], f16, tag="oh")
                for t in range(T):
                    nc.scalar.activation(
                        oh[:, t, :], out_sb[:, t, :], mybir.ActivationFunctionType.Copy
                    )
                pooled_sb = sb.tile([128, 2, F], f32, tag="pooled_sb")
                for q in range(2):
                    ppool = ps1.tile([128, F], f32, tag="ppool")
                    for t in range(T):
                        nc.tensor.matmul(
                            ppool[:],
                            R[:, t, q * 128:(q + 1) * 128],
                            oh[:, t, :],
                            start=(t == 0),
                            stop=(t == T - 1),
                        )
                    nc.scalar.activation(
                        pooled_sb[:, q, :], ppool[:], mybir.ActivationFunctionType.Copy
                    )
                nc.scalar.dma_start(
                    pooled_d.ap()[g * K:(g + 1) * K, :].rearrange("(q p) c -> p q c", p=128),
                    pooled_sb[:],
                )

                # ---- readout mean = column sums of pooled / K ----
                pmean = ps1.tile([1, F], f32, tag="misc")
                for q in range(2):
                    nc.tensor.matmul(
                        pmean[:],
                        ones_col[:],
                        pooled_sb[:, q, :],
                        start=(q == 0),
                        stop=(q == 1),
                    )
                mrow = sb.tile([1, F], f32, tag="mrow")
                nc.scalar.activation(
                    mrow[:], pmean[:], mybir.ActivationFunctionType.Copy, scale=1.0 / K
                )
                nc.gpsimd.dma_start(readout_d.ap()[g, 0:F].rearrange("(o f) -> o f", o=1), mrow[:])

                # ---- readout max: PE-transpose pooled then reduce over rows ----
                for cb in range(2):
                    mxc = sb.tile([128, 2], f32, tag="mxc")
                    for q in range(2):
                        ptr = ps1.tile([128, 128], f32, tag="misc")
                        nc.tensor.transpose(
                            ptr[:], pooled_sb[:, q, cb * 128:(cb + 1) * 128], ident[:]
                        )
                        nc.vector.reduce_max(
                            mxc[:, q:q + 1], ptr[:], axis=mybir.AxisListType.X
                        )
                    mx = sb.tile([128, 1], f32, tag="mx")
                    nc.vector.tensor_max(mx[:], mxc[:, 0:1], mxc[:, 1:2])
                    nc.gpsimd.dma_start(
                        readout_d.ap()[g, F + cb * 128:F + (cb + 1) * 128]
                        .rearrange("(p o) -> p o", o=1),
                        mx[:],
                    )
    return nc


def _jax_invsqrt(deg):
    """Bit-match the reference's jnp.power(clip(deg,1), -0.5) on CPU."""
    try:
        import jax
        cpu = jax.local_devices(backend="cpu")[0]
        with jax.default_device(cpu):
            import jax.numpy as jnp
            return np.asarray(
                jnp.power(jnp.clip(jax.device_put(deg, cpu), 1.0), -0.5), np.float32
            )
    except Exception:
        return (np.clip(deg, 1.0, None) ** -0.5).astype(np.float32)


_NC_CACHE = {}


def _get_nc():
    if "nc" not in _NC_CACHE:
        nc = bacc.Bacc("TRN2", target_bir_lowering=False, debug=False, num_devices=NCORES)
        _build(nc)
        nc.compile()
        _NC_CACHE["nc"] = nc
    return _NC_CACHE["nc"]


def kernel(feat, e_feat, W, b, src, dst):
    feat = np.asarray(feat, np.float32)
    e_feat = np.asarray(e_feat, np.float32)
    W = np.asarray(W, np.float32)
    b = np.asarray(b, np.float32)
    src = np.asarray(src)
    dst = np.asarray(dst)
    N = B * NP

    out_deg = np.bincount(src, minlength=N).astype(np.float32)
    in_deg = np.bincount(dst, minlength=N).astype(np.float32)
    sn = _jax_invsqrt(out_deg)
    dn = _jax_invsqrt(in_deg)

    # dense transposed adjacency per graph: At[g*512+s, d] = sum of e_feat over edges (s->d)
    g = src // NP
    lin = (g.astype(np.int64) * NP + (src % NP)) * NP + (dst % NP)
    At_full = np.bincount(lin, weights=e_feat.astype(np.float64), minlength=B * NP * NP)
    At_full = At_full.reshape(B * NP, NP).astype(np.float32)

    featT = np.ascontiguousarray(feat.T)

    nc = _get_nc()
    in_maps = []
    for c in range(NCORES):
        nsl = slice(c * NNC, (c + 1) * NNC)
        in_maps.append({
            "featT": np.ascontiguousarray(featT[:, nsl]),
            "At": np.ascontiguousarray(At_full[nsl]),
            "sn": np.ascontiguousarray(sn[nsl]),
            "dn": np.ascontiguousarray(dn[nsl]),
            "W": W,
            "b": b,
        })
    trace = bool(int(os.environ.get("KERNEL_TRACE", "0")))
    if trace:
        try:
            import antenv.axon_hooks  # noqa: F401
        except ImportError:
            import sys
            import types
            from trn_agent_boot.trn_boot import _ntff_profile_via_ctypes
            _h = _ntff_profile_via_ctypes("/opt/axon/libaxon_pjrt.so")
            mod = types.ModuleType("antenv.axon_hooks")
            mod.get_axon_ntff_profile_hook = lambda: _h
            mod.set_axon_ntff_profile_hook = lambda hook: None
            sys.modules["antenv.axon_hooks"] = mod
    res = run_bass_kernel_spmd(
        nc, in_maps, core_ids=list(range(NCORES)), trace=trace,
        tmpdir=os.environ.get("KERNEL_TRACE_DIR") or None,
    )
    if trace and res.exec_time_ns is not None:
        print(f"HW exec time: {res.exec_time_ns} ns")
    outs = res.results
    pooled = np.concatenate([outs[c]["pooled"] for c in range(NCORES)], axis=0)
    readout = np.concatenate([outs[c]["readout"] for c in range(NCORES)], axis=0)
    return pooled, readout
